# revision 15
# baseline (speedup 1.0000x reference)
"""Trainium2 Bass kernel for DFMN prototypical-network loss (retrieval_knn).

Reference math (per pixel, 64-way episode):
    protos = prototypes[indices]                         # [64, C]
    logits = -(|q|^2 + |p|^2 - 2 q.p)                    # [N, 64]
    loss   = -mean(log_softmax(logits)[label])

Key identity: the per-pixel |q|^2 term is constant across classes, so it
cancels in log_softmax.  With G = q.p and z = 2G - pn (pn = |p|^2 per class):
    -logp[label] = LSE_c(z) - z[label]
    loss = [ sum_px ln(sum_c e^z) - sum_img (2*rowsumG[label_b] - 196*pn[label_b]) ] / N

Device layout per core (64 images = 32 image pairs, data-parallel over 8
cores).  The q stream is fp8e4 (quantization costs ~7e-4 rel err vs the
2e-2 budget) which halves HBM bytes to ~12.9 MB/core — the stream runs at
~352 GB/s, pinned at the per-core HBM roofline.  fp8 alone would leave the
PE the bottleneck (fp8 matmul runs at fp16 speed), so the matmuls use
perf_mode=DoubleRow: 2 fp8 weights per PE cell, contraction K=256 per
instruction at 0.5 cycles/row — PE main work halves to ~24 us < stream.
DoubleRow is mutually exclusive with PE column tiling, so each pair owns a
[64, 392] PSUM bank (6 rotating banks), and work is pipelined in 16
two-pair mini-groups so the PE never lags the stream by more than one
mini-group (~2 us):
  - G via TensorE:   gps_j[64, 392] = sum_kk protos2T.T @ Q2  (4 DoubleRow
    steps, weights [128,2,64], moving [128,2,392]; per-kk LDWEIGHTS shared
    by the mini-group's pairs, redundant reloads deduplicated)
  - exp via ScalarE: e_j = Exp(2*G - pn)  (PSUM -> SBUF bf16, fused
    scale/bias; Exp is the only ACT table fn -> no table swaps)
  - colsum via TensorE: s[32, 392] += sel.T @ e_j (ones-column selector,
    accumulated over pairs 0..29; shipped to HBM while the tail computes)
  - label rowsums via VectorE: r[64, 2] per pair = 3D X-reduce of G
  - tail (pairs 30/31) streams pair-a whole + pair-b in half-k chunks so
    only ~4 matmuls + exp + a 2-row selector + 3 KB ship-out trail the
    stream; their colsums land in a separate tiny PSUM bank.
Host finishes: ln() of the shipped colsums (float64), label gather from r,
pn terms, exact mean.
"""

import sys

for _p in ("/opt/trn_rl_repo",):
    if _p not in sys.path:
        sys.path.insert(0, _p)

import numpy as np

import concourse.bass as bass
import concourse.bacc as bacc
import concourse.tile as tile
from concourse import mybir
from concourse.bass_utils import run_bass_kernel_spmd

# Problem constants (nn_DFMNLoss: B=512, C=1024, 14x14 features, 64-way)
B = 512
C = 1024
F2 = 196          # 14 * 14 pixels per image
NWAY = 64
NCORES = 8
BPC = B // NCORES           # 64 images per core
NPAIR = BPC // 2            # 32 image pairs per core
NMINI = NPAIR // 2          # 16 two-pair pipeline mini-groups
KT = C // 128               # 8 contraction chunks of 128 channels
PAIRCOLS = 2 * F2           # 392 pixel columns per pair
QCOLS = KT * PAIRCOLS       # 3136 elements per partition per pair

F32 = mybir.dt.float32
F8 = mybir.dt.float8e4
F8_NP = mybir.dt.np(F8)
BF16 = mybir.dt.bfloat16
BF16_NP = mybir.dt.np(BF16)
DR = mybir.MatmulPerfMode.DoubleRow

_CACHE = {}


def _dedup_ldweights(nc):
    """Drop InstLdweights that reload weights already resident in the PE
    array.  Tile emits one LDWEIGHTS per matmul; inside a mini-group the
    same prototype chunk is loaded for both pairs, and the loads serialize
    with the matmul stream on the PE.  Matmults here are non-self-loading,
    so a dropped reload just reuses the array contents.  Tracks state per
    32-wide PE column group; only sync-free LDWs are dropped, and any
    dangling dependency names are remapped to the keeper.
    """
    removed = {}
    for blk in nc.m.functions[0].blocks:
        state = {}  # col_group -> (key, keeper_name)
        kept = []
        for inst in blk.instructions:
            if isinstance(inst, mybir.InstLdweights):
                a = inst.ins[0]
                tp = inst.tile_position or (0, 0)
                ts = inst.tile_size or (128, 128)
                key = (a.memref, a.offset, str(a.ap), str(a.dtype), tp, ts,
                       inst.perf_mode)
                cgs = range(tp[1] // 32, (tp[1] + ts[1] + 31) // 32)
                si = inst.sync_info
                clean = si is None or (not si.on_wait and not si.on_update)
                prev = [state.get(cg) for cg in cgs]
                if clean and all(p is not None and p[0] == key for p in prev):
                    removed[inst.name] = prev[0][1]
                    continue
                for cg in cgs:
                    state[cg] = (key, inst.name)
            kept.append(inst)
        blk.instructions[:] = kept
    if removed:
        for blk in nc.m.functions[0].blocks:
            for inst in blk.instructions:
                names = set(inst.sync_dependency_names()) | set(
                    inst.nosync_dependency_names()
                )
                if names & removed.keys():
                    inst.remap_dependency_names(
                        {k: v for k, v in removed.items() if k in names}
                    )
        for k in removed:
            nc.inst_map.pop(k, None)
    return len(removed)


def _build_nc():
    # Bacc (not raw Bass): its compile() pass splits multi-wait instructions
    # into event semaphores — walrus allows only one sync wait per instruction.
    nc = bacc.Bacc()
    q = nc.dram_tensor("q", [NPAIR * 128, QCOLS], F8, kind="ExternalInput")
    pT = nc.dram_tensor("pT", [128, KT * NWAY], F8, kind="ExternalInput")
    negpn = nc.dram_tensor("negpn", [NWAY, 1], F32, kind="ExternalInput")
    bsel2 = nc.dram_tensor(
        "bsel2", [NWAY, 2 * NPAIR - 1], BF16, kind="ExternalInput"
    )
    rsum = nc.dram_tensor("rsum", [NWAY, 2 * NPAIR], F32, kind="ExternalOutput")
    sexp = nc.dram_tensor("sexp", [NPAIR, PAIRCOLS], F32, kind="ExternalOutput")
    sexpb = nc.dram_tensor("sexpb", [2, PAIRCOLS], F32, kind="ExternalOutput")

    with tile.TileContext(nc) as tc:
        with (
            tc.tile_pool(name="const", bufs=1) as cpool,
            tc.tile_pool(name="qin", bufs=6) as qpool,
            tc.tile_pool(name="qtail", bufs=1) as tpool,
            tc.tile_pool(name="acc", bufs=1) as apool,
            tc.tile_pool(name="gps", bufs=6, space="PSUM") as gpool,
            tc.tile_pool(name="sps", bufs=1, space="PSUM") as spool,
        ):
            def mini_dma(g):
                t = qpool.tile([128, 2 * QCOLS], F8, name="gt", tag="gt")
                nc.sync.dma_start(
                    t[:, 0 : 2 * QCOLS].rearrange("p (j c) -> p j c", c=QCOLS),
                    q[2 * g * 128 : (2 * g + 2) * 128, :].rearrange(
                        "(j p) c -> p j c", p=128
                    ),
                )
                return t

            # First mini DMA is issued before the const DMAs so the big HBM
            # stream starts as early as possible; the constants land while
            # the first mini-group is still in flight.
            gt0 = mini_dma(0)

            p_sb = cpool.tile([128, KT * NWAY], F8)
            nc.sync.dma_start(p_sb[:], pT[:])
            npn_sb = cpool.tile([NWAY, 1], F32)
            nc.sync.dma_start(npn_sb[:], negpn[:])
            bsel_sb = cpool.tile([NWAY, 2 * NPAIR - 1], BF16)
            nc.sync.dma_start(bsel_sb[:], bsel2[:])

            r_sb = apool.tile([NWAY, 2 * NPAIR], F32)
            s_sb = apool.tile([NPAIR, PAIRCOLS], F32)
            sb_sb = apool.tile([2, PAIRCOLS], F32)
            e_all = apool.tile([NWAY, NPAIR * PAIRCOLS], BF16)
            s_ps = spool.tile([NPAIR, PAIRCOLS], F32, name="sps", tag="sps")
            s_psb = spool.tile([2, PAIRCOLS], F32, name="spsb", tag="spsb")

            # ACT warmup: absorb the npn DMA wait, the const-AP init wait and
            # the exp table load outside the hot loop.
            warm_a = cpool.tile([NWAY, 1], F32)
            warm_b = cpool.tile([NWAY, 1], F32)
            nc.scalar.copy(warm_a[:], npn_sb[:])
            nc.scalar.activation(
                warm_b[:], warm_a[:], mybir.ActivationFunctionType.Exp
            )

            def wk2_of(kk):
                return p_sb[
                    :, 2 * kk * NWAY : (2 * kk + 2) * NWAY
                ].rearrange("p (two m) -> p two m", two=2)

            def dr_matmul(gps_j, kk, rhs2):
                nc.tensor.matmul(
                    gps_j[:],
                    wk2_of(kk),
                    rhs2,
                    tile_position=(0, 0),
                    perf_mode=DR,
                    start=(kk == 0),
                    stop=(kk == KT // 2 - 1),
                    skip_group_check=True,
                )

            def sel_pair(j):
                # s_ps[j, :] += colsum over class partitions of e(pair j);
                # pairs 0..29 accumulate here (shipped while the tail runs).
                nc.tensor.matmul(
                    s_ps[:],
                    bsel_sb[:, NPAIR - 1 - j : 2 * NPAIR - 1 - j],
                    e_all[:, j * PAIRCOLS : (j + 1) * PAIRCOLS],
                    start=(j == 0),
                    stop=(j == NPAIR - 3),
                    skip_group_check=True,
                )

            def pair_post(j, gps_j):
                nc.scalar.activation(
                    e_all[:, j * PAIRCOLS : (j + 1) * PAIRCOLS],
                    gps_j[:],
                    mybir.ActivationFunctionType.Exp,
                    bias=npn_sb[:],
                    scale=2.0,
                )
                nc.vector.reduce_sum(
                    r_sb[:, 2 * j : 2 * j + 2],
                    gps_j[:].rearrange("p (i f) -> p i f", f=F2),
                    axis=mybir.AxisListType.X,
                )

            for g in range(NMINI - 1):      # mini-groups 0..14, pairs 0..29
                gt = gt0 if g == 0 else mini_dma(g)
                gps = {
                    jl: gpool.tile(
                        [NWAY, PAIRCOLS], F32, name="gps", tag="gps"
                    )
                    for jl in range(2)
                }
                for kk in range(KT // 2):
                    for jl in range(2):
                        dr_matmul(
                            gps[jl],
                            kk,
                            gt[
                                :,
                                jl * QCOLS
                                + 2 * kk * PAIRCOLS : jl * QCOLS
                                + (2 * kk + 2) * PAIRCOLS,
                            ].rearrange("p (two c) -> p two c", c=PAIRCOLS),
                        )
                # Selector matmuls lag one mini-group so the PE never stalls
                # on the ACT exp (exp(g-1) ran during this group's matmuls).
                if g > 0:
                    sel_pair(2 * g - 2)
                    sel_pair(2 * g - 1)
                pair_post(2 * g, gps[0])
                pair_post(2 * g + 1, gps[1])

            # ---- serial tail: pairs 30 and 31 ----
            # Pair-a streams whole (3136 B descriptor runs), pair-b in two
            # half-k chunks (1568 B runs) — descriptors stay fat so the tail
            # rides the dense HBM stream instead of trailing it.  Only
            # pair-b's last matmuls, its exp, a 2-row selector and a 3 KB
            # ship-out remain after the stream ends.
            ta = tpool.tile([128, QCOLS], F8, name="ta", tag="ta")
            nc.sync.dma_start(ta[:], q[30 * 128 : 31 * 128, :])
            tb = []
            for hi in range(2):
                h = tpool.tile(
                    [128, QCOLS // 2], F8, name=f"tb{hi}", tag=f"tb{hi}"
                )
                nc.sync.dma_start(
                    h[:],
                    q[
                        31 * 128 : 32 * 128,
                        hi * (QCOLS // 2) : (hi + 1) * (QCOLS // 2),
                    ],
                )
                tb.append(h)
            gps30 = gpool.tile([NWAY, PAIRCOLS], F32, name="gps", tag="gps")
            gps31 = gpool.tile([NWAY, PAIRCOLS], F32, name="gps", tag="gps")
            for kk in range(KT // 2):
                dr_matmul(
                    gps30,
                    kk,
                    ta[
                        :, 2 * kk * PAIRCOLS : (2 * kk + 2) * PAIRCOLS
                    ].rearrange("p (two c) -> p two c", c=PAIRCOLS),
                )
            sel_pair(28)
            sel_pair(29)
            pair_post(30, gps30)
            # s for pairs 0..29 ships while pair-b computes.
            nc.scalar.copy(s_sb[:], s_ps[:])
            nc.sync.dma_start(rsum[:, 0:60], r_sb[:, 0:60])
            nc.sync.dma_start(sexp[:], s_sb[:])
            for kk in range(KT // 2):
                src = tb[0] if kk < 2 else tb[1]
                base = (kk % 2) * 2 * PAIRCOLS
                dr_matmul(
                    gps31,
                    kk,
                    src[:, base : base + 2 * PAIRCOLS].rearrange(
                        "p (two c) -> p two c", c=PAIRCOLS
                    ),
                )
            pair_post(31, gps31)
            # Tail pairs' colsums land in their own tiny 2-row PSUM bank.
            nc.tensor.matmul(
                s_psb[:],
                bsel_sb[:, NPAIR - 1 : NPAIR + 1],
                e_all[:, 30 * PAIRCOLS : 31 * PAIRCOLS],
                start=True,
                stop=False,
                skip_group_check=True,
            )
            nc.tensor.matmul(
                s_psb[:],
                bsel_sb[:, NPAIR - 2 : NPAIR],
                e_all[:, 31 * PAIRCOLS : 32 * PAIRCOLS],
                start=False,
                stop=True,
                skip_group_check=True,
            )
            nc.scalar.copy(sb_sb[:], s_psb[:])
            nc.sync.dma_start(rsum[:, 60:64], r_sb[:, 60:64])
            nc.sync.dma_start(sexpb[:], sb_sb[:])

    n = _dedup_ldweights(nc)
    if n < 30:
        print(f"[kernel] warning: ldweights dedup removed only {n}", flush=True)
    nc.compile()
    return nc


def _get_nc():
    if "nc" not in _CACHE:
        _CACHE["nc"] = _build_nc()
    return _CACHE["nc"]


def _pack_core_q(qc32):
    # [64, C, F2] -> [pair, p, k, i, f] -> [NPAIR*128, QCOLS] fp8e4
    qc = qc32.reshape(NPAIR, 2, KT, 128, F2).transpose(0, 3, 2, 1, 4)
    return np.ascontiguousarray(qc, dtype=F8_NP).reshape(NPAIR * 128, QCOLS)


def _prepare(query_features, labels, prototypes, indices):
    """Returns (in_maps, labels_i64, pn32)."""
    qf = np.asarray(query_features, dtype=np.float32).reshape(B, C, F2)
    labels = np.asarray(labels).astype(np.int64)
    protos = np.asarray(prototypes, dtype=np.float32)
    idx = np.asarray(indices).astype(np.int64)

    pg = protos[idx]                                     # [64, C] fp32
    pn32 = np.sum(pg.astype(np.float64) ** 2, axis=1).astype(np.float32)
    negpn_np = np.ascontiguousarray((-pn32).reshape(NWAY, 1))
    pT_pack = np.ascontiguousarray(
        pg.T.reshape(KT, 128, NWAY).transpose(1, 0, 2), dtype=F8_NP
    ).reshape(128, KT * NWAY)
    bsel2_np = np.zeros((NWAY, 2 * NPAIR - 1), dtype=BF16_NP)
    bsel2_np[:, NPAIR - 1] = 1

    in_maps = [
        {
            "q": _pack_core_q(qf[c * BPC : (c + 1) * BPC]),
            "pT": pT_pack,
            "negpn": negpn_np,
            "bsel2": bsel2_np,
        }
        for c in range(NCORES)
    ]
    return in_maps, labels, pn32


def kernel(query_features, labels, prototypes, indices, n_way):
    import time as _time

    t0 = _time.time()
    nc = _get_nc()
    t1 = _time.time()
    in_maps, labels, pn32 = _prepare(query_features, labels, prototypes, indices)
    t2 = _time.time()
    results = run_bass_kernel_spmd(nc, in_maps, list(range(NCORES))).results
    t3 = _time.time()
    print(
        f"[kernel] build={t1 - t0:.1f}s pack={t2 - t1:.1f}s run={t3 - t2:.1f}s",
        flush=True,
    )

    # Host-side finish: r[64, 64] holds per-image rowsums of G at
    # [class, local_image]; s tiles hold per-pixel sum_c exp(2G - pn).
    pn64 = pn32.astype(np.float64)
    larr = np.arange(BPC)
    total_lse = 0.0
    label_term = 0.0
    for c in range(NCORES):
        total_lse += float(
            np.log(results[c]["sexp"][0:30].astype(np.float64)).sum()
        ) + float(np.log(results[c]["sexpb"].astype(np.float64)).sum())
        r2 = results[c]["rsum"].astype(np.float64)       # [64, 64]
        lab = labels[c * BPC : (c + 1) * BPC]
        label_term += float(
            np.sum(2.0 * r2[lab, larr] - F2 * pn64[lab])
        )
    loss = (total_lse - label_term) / (B * F2)
    return np.asarray(loss, dtype=np.float32)
